# revision 1
# baseline (speedup 1.0000x reference)
"""Trainium2 Bass kernel for an attention block with softmax over the QUERY axis.

Reference computation (per batch b):
    Q = x_b @ Wq^T + bq ; K = x_b @ Wk^T + bk ; V = x_b @ Wv^T + bv
    S = Q @ K^T / sqrt(256)
    attn = softmax(S, axis over queries)      # couples rows, not columns
    out_b = attn @ V

Sharding over 8 NeuronCores: core m handles batch b = m // 2 and the
key/value half h = m % 2 (keys [h*2048, (h+1)*2048)).  Each core holds the
full query range for its batch, so the softmax over queries is fully local
(it normalizes each *key* row of S^T over all queries).  Each core produces
a partial output (sum over its 2048 keys); the host adds the two partials
per batch.  No collectives are required.

On-core layout (all matmul operands bf16, fp32 accumulation):
    xT   (256d, 4096s)  - host-transposed input, d on partitions
    QT   (256e, 4096q)  = Wq @ x^T   (projection output, e on partitions)
    KT   (256e, 2048k)  = Wk @ x_h^T
    V    (2048k, 256e)  = x_h @ Wv^T (natural layout, k on partitions)
    ST   (k, q) tiles   = KT^T slices @ QT   (scores transposed)
    e    = exp(ST/16)   (no max-subtraction needed: |S/16| < ~2.2)
    s_k  = sum_q e[k,q] (free-axis accumulate inside the Exp activation)
    V'   = V / s_k      (per-partition scale)
    out  = e^T @ V'     (accumulated over the 16 key tiles)
"""

import numpy as np
import ml_dtypes

import concourse.bass as bass
import concourse.tile as tile
from concourse import bacc, mybir
from concourse.bass_utils import run_bass_kernel_spmd

BF16 = ml_dtypes.bfloat16
F32 = mybir.dt.float32
BF = mybir.dt.bfloat16

B, S, D = 4, 4096, 256
NCORES = 8
KH = S // 2          # 2048 keys per core
NKT = KH // 128      # 16 key tiles
NQT = S // 128       # 32 query tiles
SLOT = 2048          # scores psum slot (4 banks) -> one Exp per slot
NSL = S // SLOT      # 2 slots per key tile
NGR = SLOT // 512    # 4 matmul groups per slot


def _emit(tc, xT, xTh, wqT, wkT, wvT, bqc, bkc, bvr, out):
    nc = tc.nc
    EXP = mybir.ActivationFunctionType.Exp
    IDENT = mybir.ActivationFunctionType.Identity
    AX = mybir.AxisListType.X

    with tc.tile_pool(name="const", bufs=1) as cpool, \
         tc.tile_pool(name="big", bufs=1) as bpool, \
         tc.tile_pool(name="work", bufs=4) as wpool:

        # ---- input loads ----
        xT_sb = []
        xTh_sb = []
        for i in range(2):
            xt = cpool.tile([128, S], BF, name=f"xTsb{i}", tag=f"xTsb{i}")
            nc.sync.dma_start(xt, xT[128 * i:128 * (i + 1), :])
            xT_sb.append(xt)
            xh = cpool.tile([128, KH], BF, name=f"xThsb{i}", tag=f"xThsb{i}")
            nc.sync.dma_start(xh, xTh[128 * i:128 * (i + 1), :])
            xTh_sb.append(xh)

        def load_w(name, src):
            ts = []
            for i in range(2):
                t = cpool.tile([128, D], BF, name=f"{name}{i}", tag=f"{name}{i}")
                nc.sync.dma_start(t, src[128 * i:128 * (i + 1), :])
                ts.append(t)
            return ts

        wq_sb = load_w("wq", wqT)
        wk_sb = load_w("wk", wkT)
        wv_sb = load_w("wv", wvT)

        def load_bcol(name, src):
            ts = []
            for i in range(2):
                t = cpool.tile([128, 1], F32, name=f"{name}{i}", tag=f"{name}{i}")
                nc.sync.dma_start(t, src[128 * i:128 * (i + 1), :])
                ts.append(t)
            return ts

        bq_sb = load_bcol("bq", bqc)
        bk_sb = load_bcol("bk", bkc)
        bv_sb = cpool.tile([1, D], BF, name="bv", tag="bv")
        nc.sync.dma_start(bv_sb, bvr)
        ones = cpool.tile([1, 128], BF, name="ones", tag="ones")
        nc.vector.memset(ones, 1.0)

        # ---- persistent big tiles ----
        QT_sb = [bpool.tile([128, S], BF, name=f"QT{i}", tag=f"QT{i}")
                 for i in range(2)]
        KT_sb = [bpool.tile([128, KH], BF, name=f"KT{i}", tag=f"KT{i}")
                 for i in range(2)]
        Vr_sb = [bpool.tile([128, D], BF, name=f"Vr{k}", tag=f"Vr{k}")
                 for k in range(NKT)]
        Vp_sb = [bpool.tile([128, D], BF, name=f"Vp{k}", tag=f"Vp{k}")
                 for k in range(NKT)]
        e_sb = [bpool.tile([128, S], BF, name=f"e{k}", tag=f"e{k}")
                for k in range(NKT)]

        # ---- phase 0: projections ----
        with tc.tile_pool(name="ps0", bufs=8, space="PSUM") as ps0:
            # QT[e, q] and KT[e, k] (e on partitions), bias per partition.
            for dst, w_sb, b_sb, src, ncols in (
                (QT_sb, wq_sb, bq_sb, xT_sb, S),
                (KT_sb, wk_sb, bk_sb, xTh_sb, KH),
            ):
                for i in range(2):
                    for qb in range(ncols // 512):
                        pt = ps0.tile([128, 512], F32, name="ps0t", tag="ps0t")
                        nc.tensor.matmul(
                            pt, w_sb[0][:, 128 * i:128 * (i + 1)],
                            src[0][:, 512 * qb:512 * (qb + 1)],
                            start=True, stop=False)
                        nc.tensor.matmul(
                            pt, w_sb[1][:, 128 * i:128 * (i + 1)],
                            src[1][:, 512 * qb:512 * (qb + 1)],
                            start=False, stop=True)
                        dsl = dst[i][:, 512 * qb:512 * (qb + 1)]
                        if i == 0:
                            nc.vector.tensor_scalar_add(dsl, pt, b_sb[i])
                        else:
                            nc.scalar.activation(dsl, pt, IDENT, bias=b_sb[i])
            # V natural layout (k on partitions); bias via rank-1 ones matmul.
            for k in range(NKT):
                pt = ps0.tile([128, 512], F32, name="ps0t", tag="ps0t")
                po = pt[:, 0:D]
                nc.tensor.matmul(po, xTh_sb[0][:, 128 * k:128 * (k + 1)],
                                 wv_sb[0], start=True, stop=False)
                nc.tensor.matmul(po, xTh_sb[1][:, 128 * k:128 * (k + 1)],
                                 wv_sb[1], start=False, stop=False)
                nc.tensor.matmul(po, ones, bv_sb, start=False, stop=True)
                nc.vector.tensor_copy(Vr_sb[k], po)

        # ---- phase 1: scores, exp, row sums, V scaling ----
        with tc.tile_pool(name="ps1", bufs=2, space="PSUM") as ps1:
            for k in range(NKT):
                sparts = wpool.tile([128, NSL], F32, name="sparts", tag="sparts")
                for hs in range(NSL):
                    pt = ps1.tile([128, SLOT], F32, name="ps1t", tag="ps1t")
                    for g in range(NGR):
                        q0 = SLOT * hs + 512 * g
                        sl = pt[:, 512 * g:512 * (g + 1)]
                        nc.tensor.matmul(
                            sl, KT_sb[0][:, 128 * k:128 * (k + 1)],
                            QT_sb[0][:, q0:q0 + 512], start=True, stop=False)
                        nc.tensor.matmul(
                            sl, KT_sb[1][:, 128 * k:128 * (k + 1)],
                            QT_sb[1][:, q0:q0 + 512], start=False, stop=True)
                    nc.scalar.activation(
                        e_sb[k][:, SLOT * hs:SLOT * (hs + 1)], pt, EXP,
                        scale=1.0 / 16.0, accum_out=sparts[:, hs:hs + 1])
                ssum = wpool.tile([128, 1], F32, name="ssum", tag="ssum")
                nc.vector.reduce_sum(ssum, sparts, axis=AX)
                rs = wpool.tile([128, 1], F32, name="rs", tag="rs")
                nc.vector.reciprocal(rs, ssum)
                nc.vector.tensor_scalar_mul(Vp_sb[k], Vr_sb[k], rs)

        # ---- phase 2: out[q, d] = sum_k e[k, q] * V'[k, d] ----
        with tc.tile_pool(name="ps2", bufs=8, space="PSUM") as ps2:
            for j in range(NQT):
                pt = ps2.tile([128, 512], F32, name="ps2t", tag="ps2t")
                po = pt[:, 0:D]
                for k in range(NKT):
                    nc.tensor.matmul(po, e_sb[k][:, 128 * j:128 * (j + 1)],
                                     Vp_sb[k], start=(k == 0),
                                     stop=(k == NKT - 1))
                ot = wpool.tile([128, D], F32, name="osb", tag="osb")
                nc.vector.tensor_copy(ot, po)
                nc.sync.dma_start(out[128 * j:128 * (j + 1), :], ot)


def build():
    nc = bacc.Bacc("TRN2", target_bir_lowering=False, debug=False)
    xT = nc.dram_tensor("xT", [D, S], BF, kind="ExternalInput").ap()
    xTh = nc.dram_tensor("xTh", [D, KH], BF, kind="ExternalInput").ap()
    wqT = nc.dram_tensor("wqT", [D, D], BF, kind="ExternalInput").ap()
    wkT = nc.dram_tensor("wkT", [D, D], BF, kind="ExternalInput").ap()
    wvT = nc.dram_tensor("wvT", [D, D], BF, kind="ExternalInput").ap()
    bqc = nc.dram_tensor("bqc", [D, 1], F32, kind="ExternalInput").ap()
    bkc = nc.dram_tensor("bkc", [D, 1], F32, kind="ExternalInput").ap()
    bvr = nc.dram_tensor("bvr", [1, D], BF, kind="ExternalInput").ap()
    out = nc.dram_tensor("out", [S, D], F32, kind="ExternalOutput").ap()

    with tile.TileContext(nc) as tc:
        _emit(tc, xT, xTh, wqT, wkT, wvT, bqc, bkc, bvr, out)
    nc.compile()
    return nc


_NC = None


def _get_nc():
    global _NC
    if _NC is None:
        _NC = build()
    return _NC


def make_in_maps(x, Wq, bq, Wk, bk, Wv, bv):
    wq = np.ascontiguousarray(Wq.T).astype(BF16)
    wk = np.ascontiguousarray(Wk.T).astype(BF16)
    wv = np.ascontiguousarray(Wv.T).astype(BF16)
    bqc = np.asarray(bq, np.float32).reshape(D, 1)
    bkc = np.asarray(bk, np.float32).reshape(D, 1)
    bvr = np.asarray(bv).reshape(1, D).astype(BF16)
    in_maps = []
    for core in range(NCORES):
        b, h = divmod(core, 2)
        xTb = np.ascontiguousarray(np.asarray(x[b]).T).astype(BF16)
        in_maps.append({
            "xT": xTb,
            "xTh": np.ascontiguousarray(xTb[:, h * KH:(h + 1) * KH]),
            "wqT": wq, "wkT": wk, "wvT": wv,
            "bqc": bqc, "bkc": bkc, "bvr": bvr,
        })
    return in_maps


def run(x, Wq, bq, Wk, bk, Wv, bv, trace=False):
    """Run on the 8 cores; returns (full_output, BassKernelResults)."""
    nc = _get_nc()
    in_maps = make_in_maps(x, Wq, bq, Wk, bk, Wv, bv)
    res = run_bass_kernel_spmd(nc, in_maps, core_ids=list(range(NCORES)),
                               trace=trace)
    parts = [r["out"] for r in res.results]
    full = np.stack([parts[2 * b] + parts[2 * b + 1] for b in range(B)], axis=0)
    return full.astype(np.float32), res


def kernel(x, Wq, bq, Wk, bk, Wv, bv):
    full, _ = run(x, Wq, bq, Wk, bk, Wv, bv, trace=False)
    return full
